# revision 1
# baseline (speedup 1.0000x reference)
"""Trainium2 Bass kernel for nn_BasicGRUBlock: 2-layer GRU block.

  x = y + z; h1 = GRU0(x); h2 = GRU1(h1); out = y + h2 @ W_lin.T + b_lin

Sharding: data-parallel over batch across 8 cores (8 sequences/core).
Both GRU layers run fused on each core; all intermediates stay in SBUF.

Per-core program (B=8 local batch, T=4096, I=64, H=256, G=768):
  Loop over 8-step groups:
    bulk:  DMA y,z group -> x = y+z -> PE-transpose -> gx0 = xT_aug.T @ W0T
           (one K=65 matmul incl. bias row), PSUM -> SBUF reshape DMA into
           [8(b), 8(t), 768] per-step layout.
    L0 x8: gh = Whh0 @ h (2 K-chunk f32r matmuls, W streamed, h^T stationary)
           + gx_rz folded into PSUM via identity-matmul + b_hh_n via K=1
           ones-matmul; sigmoid/tanh on ACT; h update on DVE/GPSIMD;
           h -> h^T via 2 PE transposes accumulated into a group tile.
    gx1:   h1T group tile -> gx1 matmul (2 K-chunks + bias row) -> reshape.
    L1 x8: same as L0.
    final: out = h2T.T @ W_lin^T + b_lin (K=1 bias matmul) + y; DMA out.
"""

import sys

sys.path.insert(0, "/opt/trn_rl_repo")

import numpy as np

import concourse.bass as bass
import concourse.bacc as bacc_mod
import concourse.mybir as mybir
from concourse.bass import ds
from concourse.tile import TileContext

B, T_FULL, I, H, G = 64, 4096, 64, 256, 768
NCORES = 8
BL = B // NCORES  # 8 sequences per core
GRP = 8  # time steps per group
F32 = mybir.dt.float32
F32R = mybir.dt.float32r

SIG = mybir.ActivationFunctionType.Sigmoid
TANH = mybir.ActivationFunctionType.Tanh
MULT = mybir.AluOpType.mult
ADD = mybir.AluOpType.add
SUB = mybir.AluOpType.subtract


def _r(ap):
    """View an fp32 AP as float32r for full-rate PE matmul."""
    return ap.bitcast(F32R)


def build_nc(T=T_FULL, unroll=4, debug=False):
    nc = bacc_mod.Bacc()
    dbg0_d = dbg1_d = None
    if debug:
        dbg0_d = nc.declare_dram_parameter("dbg0", [BL, T, H], F32,
                                           isOutput=True)
        dbg1_d = nc.declare_dram_parameter("dbg1", [BL, T, H], F32,
                                           isOutput=True)

    y_d = nc.declare_dram_parameter("y", [BL, T, I], F32, isOutput=False)
    z_d = nc.declare_dram_parameter("z", [BL, T, I], F32, isOutput=False)
    w0T_d = nc.declare_dram_parameter("w0T", [I + 1, G], F32R, isOutput=False)
    whh0T_d = nc.declare_dram_parameter("whh0T", [128, 2, G], F32R, isOutput=False)
    bhh0n_d = nc.declare_dram_parameter("bhh0n", [1, H], F32R, isOutput=False)
    w1T_d = nc.declare_dram_parameter("w1T", [128, 2, G], F32R, isOutput=False)
    whh1T_d = nc.declare_dram_parameter("whh1T", [128, 2, G], F32R, isOutput=False)
    b1r_d = nc.declare_dram_parameter("b1r", [1, G], F32R, isOutput=False)
    bhh1n_d = nc.declare_dram_parameter("bhh1n", [1, H], F32R, isOutput=False)
    wlinT_d = nc.declare_dram_parameter("wlinT", [128, 2, I], F32R, isOutput=False)
    blr_d = nc.declare_dram_parameter("blr", [1, I], F32R, isOutput=False)
    eye_d = nc.declare_dram_parameter("eye64", [64, 64], F32, isOutput=False)
    out_d = nc.declare_dram_parameter("out", [BL, T, I], F32, isOutput=True)

    assert T % GRP == 0
    ngroups = T // GRP
    assert ngroups % unroll == 0

    with TileContext(nc) as tc:
        with (
            tc.tile_pool(name="wpool", bufs=1) as wpool,
            tc.tile_pool(name="gx0pool", bufs=2) as gx0pool,
            tc.tile_pool(name="gx1pool", bufs=2) as gx1pool,
            tc.tile_pool(name="iopool", bufs=4) as iopool,
            tc.tile_pool(name="hgrp", bufs=2) as hgrp,
            tc.tile_pool(name="gatepool", bufs=4) as gatepool,
            tc.tile_pool(name="dbgpool", bufs=1) as dbgpool,
            tc.tile_pool(name="ps_rz", bufs=2, space="PSUM") as ps_rz,
            tc.tile_pool(name="ps_n", bufs=2, space="PSUM") as ps_n,
            tc.tile_pool(name="ps_tp", bufs=2, space="PSUM") as ps_tp,
            tc.tile_pool(name="ps_gx", bufs=2, space="PSUM") as ps_gx,
        ):
            # ---- persistent weights / constants ----
            w0T_t = wpool.tile([I + 1, G], F32R)
            whh0T_t = wpool.tile([128, 2, G], F32R)
            bhh0n_t = wpool.tile([1, H], F32R)
            w1T_t = wpool.tile([128, 2, G], F32R)
            whh1T_t = wpool.tile([128, 2, G], F32R)
            b1r_t = wpool.tile([1, G], F32R)
            bhh1n_t = wpool.tile([1, H], F32R)
            wlinT_t = wpool.tile([128, 2, I], F32R)
            blr_t = wpool.tile([1, I], F32R)
            eye_t = wpool.tile([64, 64], F32)
            onesf_t = wpool.tile([1, 128], F32)
            ones_t = wpool.tile([1, 128], F32R)
            zerof_t = wpool.tile([128, 2 * BL], F32)
            h_a = wpool.tile([BL, H], F32)  # layer-0 hidden state
            h_b = wpool.tile([BL, H], F32)  # layer-1 hidden state

            nc.sync.dma_start(out=w0T_t, in_=w0T_d[:])
            nc.sync.dma_start(out=whh0T_t, in_=whh0T_d[:])
            nc.sync.dma_start(out=bhh0n_t, in_=bhh0n_d[:])
            nc.sync.dma_start(out=w1T_t, in_=w1T_d[:])
            nc.sync.dma_start(out=whh1T_t, in_=whh1T_d[:])
            nc.sync.dma_start(out=b1r_t, in_=b1r_d[:])
            nc.sync.dma_start(out=bhh1n_t, in_=bhh1n_d[:])
            nc.sync.dma_start(out=wlinT_t, in_=wlinT_d[:])
            nc.sync.dma_start(out=blr_t, in_=blr_d[:])
            nc.sync.dma_start(out=eye_t, in_=eye_d[:])
            nc.gpsimd.memset(onesf_t[:], 1.0)
            nc.gpsimd.memset(zerof_t[:], 0.0)
            nc.vector.tensor_copy(ones_t[:], onesf_t[:])
            nc.gpsimd.memset(h_a[:], 0.0)
            nc.gpsimd.memset(h_b[:], 0.0)

            # persistent carry of last step's transposed h per layer
            hT7a = wpool.tile([128, 2, BL], F32R)
            hT7b = wpool.tile([128, 2, BL], F32R)
            i8r = wpool.tile([8, 8], F32R)
            nc.vector.tensor_copy(hT7a[:, 0, :], zerof_t[:, 0:BL])
            nc.vector.tensor_copy(hT7a[:, 1, :], zerof_t[:, BL:2 * BL])
            nc.vector.tensor_copy(hT7b[:, 0, :], zerof_t[:, 0:BL])
            nc.vector.tensor_copy(hT7b[:, 1, :], zerof_t[:, BL:2 * BL])
            nc.vector.tensor_copy(i8r, eye_t[0:8, 0:8])


            def recur_step(k, hT_prev, hT_cur, h_s, whhT_t, bhn_t, gx_t):
                """One GRU step for one layer. hT_prev: persistent [128,2,8]
                carry tile; hT_cur: [128,2,64] group tile; h_s: [8,H] state."""
                if k == 0:
                    hT0 = hT_prev[:, 0, :]  # [128, 8]
                    hT1 = hT_prev[:, 1, :]
                else:
                    hT0 = hT_cur[:, 0, (k - 1)::8]  # stride-8 column slice
                    hT1 = hT_cur[:, 1, (k - 1)::8]

                rzP = ps_rz.tile([BL, 512], F32, tag="rz")
                nc.tensor.matmul(rzP, _r(hT0), _r(whhT_t[:, 0, 0:512]),
                                 start=True, stop=False)
                nc.tensor.matmul(rzP, _r(hT1), _r(whhT_t[:, 1, 0:512]),
                                 start=False, stop=False)
                # fold gx_rz into PSUM: out += I8.T @ gx_rz
                nc.tensor.matmul(rzP, i8r[:], _r(gx_t[:, k, 0:512]),
                                 start=False, stop=True)

                nP = ps_n.tile([BL, H], F32, tag="n")
                nc.tensor.matmul(nP, _r(hT0), _r(whhT_t[:, 0, 512:768]),
                                 start=True, stop=False)
                nc.tensor.matmul(nP, _r(hT1), _r(whhT_t[:, 1, 512:768]),
                                 start=False, stop=False)
                # fold b_hh_n into PSUM: out += ones.T @ b_hh_n
                nc.tensor.matmul(nP, _r(ones_t[:, 0:8]), _r(bhn_t),
                                 start=False, stop=True)

                rz_s = gatepool.tile([BL, 512], F32, tag="rz_s")
                nc.scalar.activation(rz_s, rzP, SIG)
                m_s = gatepool.tile([BL, H], F32, tag="m_s")
                nc.vector.tensor_tensor(m_s, rz_s[:, 0:H], nP, MULT)
                tn_s = gatepool.tile([BL, H], F32, tag="tn_s")
                nc.vector.tensor_tensor(tn_s, m_s, gx_t[:, k, 512:768].bitcast(F32), ADD)
                n_s = gatepool.tile([BL, H], F32, tag="n_s")
                nc.scalar.activation(n_s, tn_s, TANH)
                d_s = gatepool.tile([BL, H], F32, tag="d_s")
                nc.gpsimd.tensor_tensor(d_s, h_s, n_s, SUB)
                e_s = gatepool.tile([BL, H], F32, tag="e_s")
                nc.vector.tensor_tensor(e_s, rz_s[:, H:512], d_s, MULT)
                nc.vector.tensor_tensor(h_s, n_s, e_s, ADD)  # h = n + z*(h-n)

                tp = ps_tp.tile([128, 16], F32, tag="tp")
                nc.tensor.transpose(tp[:, 0:8], h_s[:, 0:128], eye_t[0:8, 0:8])
                nc.tensor.transpose(tp[:, 8:16], h_s[:, 128:256], eye_t[0:8, 0:8])
                nc.vector.tensor_copy(hT_cur[:, 0, k::8], tp[:, 0:8])
                nc.vector.tensor_copy(hT_cur[:, 1, k::8], tp[:, 8:16])

            def gx_from_hT(hT_t, wT_t, brow_t):
                """gx group matmul: [64(b*8+t), 768] = hT.T @ W^T + b."""
                p1 = ps_gx.tile([64, 512], F32, tag="gx")
                nc.tensor.matmul(p1, _r(hT_t[:, 0, :]), _r(wT_t[:, 0, 0:512]),
                                 start=True, stop=False)
                nc.tensor.matmul(p1, _r(hT_t[:, 1, :]), _r(wT_t[:, 1, 0:512]),
                                 start=False, stop=False)
                nc.tensor.matmul(p1, _r(ones_t[:, 0:64]), _r(brow_t[:, 0:512]),
                                 start=False, stop=True)
                p2 = ps_gx.tile([64, 256], F32, tag="gx")
                nc.tensor.matmul(p2, _r(hT_t[:, 0, :]), _r(wT_t[:, 0, 512:768]),
                                 start=True, stop=False)
                nc.tensor.matmul(p2, _r(hT_t[:, 1, :]), _r(wT_t[:, 1, 512:768]),
                                 start=False, stop=False)
                nc.tensor.matmul(p2, _r(ones_t[:, 0:64]), _r(brow_t[:, 512:768]),
                                 start=False, stop=True)
                return p1, p2

            def body(t0):
                # ---------- bulk: x = y + z, gx0 ----------
                y_t = iopool.tile([64, I], F32, tag="y")
                z_t = iopool.tile([64, I], F32, tag="z")
                nc.sync.dma_start(out=y_t, in_=y_d[:, ds(t0, GRP), :])
                nc.sync.dma_start(out=z_t, in_=z_d[:, ds(t0, GRP), :])
                x_t = iopool.tile([64, I], F32, tag="x")
                nc.vector.tensor_tensor(x_t, y_t, z_t, ADD)
                xp = ps_gx.tile([64, 64], F32, tag="gx")
                nc.tensor.transpose(xp, x_t, eye_t)
                xT_t = iopool.tile([I + 1, 64], F32R, tag="xT")
                nc.vector.tensor_copy(xT_t[0:I, :], xp)
                nc.vector.tensor_copy(xT_t[I : I + 1, :], onesf_t[:, 0:64])

                p1 = ps_gx.tile([64, 512], F32, tag="gx")
                nc.tensor.matmul(p1, _r(xT_t), _r(w0T_t[:, 0:512]),
                                 start=True, stop=True)
                p2 = ps_gx.tile([64, 256], F32, tag="gx")
                nc.tensor.matmul(p2, _r(xT_t), _r(w0T_t[:, 512:768]),
                                 start=True, stop=True)
                gs0 = iopool.tile([64, G], F32R, tag="gs0")
                nc.scalar.copy(gs0[:, 0:512], p1)
                nc.vector.tensor_copy(gs0[:, 512:768], p2)
                gx0_t = gx0pool.tile([BL, GRP, G], F32R, tag="gx0")
                nc.sync.dma_start(out=gx0_t, in_=gs0)

                # ---------- layer 0 ----------
                h1T_t = hgrp.tile([128, 2, 64], F32R, tag="h1T")
                dbg0_t = None
                if debug:
                    dbg0_t = dbgpool.tile([BL, GRP, H], F32, tag="dbg0")
                for k in range(GRP):
                    recur_step(k, hT7a, h1T_t, h_a, whh0T_t,
                               bhh0n_t, gx0_t)
                    if debug:
                        nc.vector.tensor_copy(dbg0_t[:, k, :], h_a)
                nc.vector.tensor_copy(hT7a[:, 0, :], h1T_t[:, 0, 7::8])
                nc.vector.tensor_copy(hT7a[:, 1, :], h1T_t[:, 1, 7::8])
                if debug:
                    nc.sync.dma_start(out=dbg0_d[:, ds(t0, GRP), :],
                                      in_=dbg0_t)

                # ---------- gx1 from h1T ----------
                q1, q2 = gx_from_hT(h1T_t, w1T_t, b1r_t)
                gs1 = iopool.tile([64, G], F32R, tag="gs1")
                nc.scalar.copy(gs1[:, 0:512], q1)
                nc.vector.tensor_copy(gs1[:, 512:768], q2)
                gx1_t = gx1pool.tile([BL, GRP, G], F32R, tag="gx1")
                nc.sync.dma_start(out=gx1_t, in_=gs1)

                # ---------- layer 1 ----------
                h2T_t = hgrp.tile([128, 2, 64], F32R, tag="h2T")
                dbg1_t = None
                if debug:
                    dbg1_t = dbgpool.tile([BL, GRP, H], F32, tag="dbg1")
                for k in range(GRP):
                    recur_step(k, hT7b, h2T_t, h_b, whh1T_t,
                               bhh1n_t, gx1_t)
                    if debug:
                        nc.vector.tensor_copy(dbg1_t[:, k, :], h_b)
                nc.vector.tensor_copy(hT7b[:, 0, :], h2T_t[:, 0, 7::8])
                nc.vector.tensor_copy(hT7b[:, 1, :], h2T_t[:, 1, 7::8])
                if debug:
                    nc.sync.dma_start(out=dbg1_d[:, ds(t0, GRP), :],
                                      in_=dbg1_t)

                # ---------- final linear + residual ----------
                f1 = ps_gx.tile([64, I], F32, tag="gx")
                nc.tensor.matmul(f1, _r(h2T_t[:, 0, :]), _r(wlinT_t[:, 0, :]),
                                 start=True, stop=False)
                nc.tensor.matmul(f1, _r(h2T_t[:, 1, :]), _r(wlinT_t[:, 1, :]),
                                 start=False, stop=False)
                nc.tensor.matmul(f1, _r(ones_t[:, 0:64]), _r(blr_t),
                                 start=False, stop=True)
                o_t = iopool.tile([64, I], F32, tag="o")
                nc.vector.tensor_tensor(o_t, f1, y_t, ADD)
                nc.sync.dma_start(out=out_d[:, ds(t0, GRP), :], in_=o_t)

            if ngroups <= unroll:
                for gi in range(ngroups):
                    body(gi * GRP)
            else:
                with tc.For_i(0, T, GRP * unroll,
                              staggered_reset=True) as iv:
                    for i in range(unroll):
                        if i > 0 and unroll == 4:
                            tc.stage_boundary()
                        body(iv + i * GRP)

    nc.compile()
    return nc


def prep_weights(W_ih0, W_hh0, b_ih0, b_hh0, W_ih1, W_hh1, b_ih1, b_hh1,
                 W_lin, b_lin):
    """Host-side weight folding. Returns dict of prepped arrays."""
    f = np.float32
    pad_rz = lambda b: np.concatenate([b[: 2 * H], np.zeros(H, f)])
    w0T = np.concatenate(
        [W_ih0.T, (b_ih0 + pad_rz(b_hh0))[None, :]], axis=0
    ).astype(f)  # [65, 768]
    whh0T = np.ascontiguousarray(
        W_hh0.T.reshape(2, 128, G).transpose(1, 0, 2)
    ).astype(f)  # [128, 2, 768]
    w1T = np.ascontiguousarray(
        W_ih1.T.reshape(2, 128, G).transpose(1, 0, 2)
    ).astype(f)
    whh1T = np.ascontiguousarray(
        W_hh1.T.reshape(2, 128, G).transpose(1, 0, 2)
    ).astype(f)
    wlinT = np.ascontiguousarray(
        W_lin.T.reshape(2, 128, I).transpose(1, 0, 2)
    ).astype(f)
    return {
        "w0T": w0T,
        "whh0T": whh0T,
        "bhh0n": b_hh0[2 * H :][None, :].astype(f),
        "w1T": w1T,
        "whh1T": whh1T,
        "b1r": (b_ih1 + pad_rz(b_hh1))[None, :].astype(f),
        "bhh1n": b_hh1[2 * H :][None, :].astype(f),
        "wlinT": wlinT,
        "blr": b_lin[None, :].astype(f),
        "eye64": np.eye(64, dtype=f),
    }


_NC_CACHE = {}


def kernel(z, y, W_ih0, W_hh0, b_ih0, b_hh0, W_ih1, W_hh1, b_ih1, b_hh1,
           W_lin, b_lin, _trace=False):
    """Full-input entry point: shards over 8 cores, returns full output."""
    from concourse.bass_utils import run_bass_kernel_spmd

    z = np.asarray(z, np.float32)
    y = np.asarray(y, np.float32)
    weights = dict(W_ih0=np.asarray(W_ih0), W_hh0=np.asarray(W_hh0),
                   b_ih0=np.asarray(b_ih0), b_hh0=np.asarray(b_hh0),
                   W_ih1=np.asarray(W_ih1), W_hh1=np.asarray(W_hh1),
                   b_ih1=np.asarray(b_ih1), b_hh1=np.asarray(b_hh1),
                   W_lin=np.asarray(W_lin), b_lin=np.asarray(b_lin))
    T = z.shape[1]
    key = T
    if key not in _NC_CACHE:
        _NC_CACHE[key] = build_nc(T=T)
    nc = _NC_CACHE[key]

    wmaps = prep_weights(**weights)
    in_maps = []
    for c in range(NCORES):
        sl = slice(c * BL, (c + 1) * BL)
        m = {
            "z": np.ascontiguousarray(z[sl]),
            "y": np.ascontiguousarray(y[sl]),
            "whh0T": wmaps["whh0T"],
            "whh1T": wmaps["whh1T"],
        }
        for k in ("w0T", "bhh0n", "w1T", "b1r", "bhh1n", "wlinT", "blr",
                  "eye64"):
            m[k] = wmaps[k]
        in_maps.append(m)

    res = run_bass_kernel_spmd(nc, in_maps, list(range(NCORES)), trace=_trace)
    outs = [res.results[c]["out"] for c in range(NCORES)]
    full = np.concatenate(outs, axis=0).astype(np.float32)
    if _trace:
        return full, res
    return full



# revision 15
# speedup vs baseline: 2.0271x; 2.0271x over previous
"""Trainium2 Bass kernel for nn_BasicGRUBlock: 2-layer GRU block.

  x = y + z; h1 = GRU0(x); h2 = GRU1(h1); out = y + h2 @ W_lin.T + b_lin

Sharding: data-parallel over batch across 8 cores (8 sequences/core).

Gate-major design: all per-step tensors live as [gates/hidden on 128
partitions, batch on free axis].  Recurrent matmuls keep the weights
stationary (bf16, 128-col tiles -> compiler FWL) and stream h^T [128, 8];
gates emerge in PSUM as [128, chunks, 8], so every element-wise op is a
wide 128-partition op with 16-32 free elements, and the hidden update
lands directly in the layout the next matmul consumes - no transposes in
the recurrent chain.

Pipeline per group of GRP=16 steps (body g):
  [L0 step k of group g ; L1 step k of group g-1] x 16   (interleaved)
  gx1(g):   W_ih1 @ H1(g) group matmuls -> G1/N1
  final(g-1): out = y + W_lin @ H2(g-1) + b_lin -> DMA
  bulk0(g+1): DMA y,z; x=y+z; x^T; W_ih0 @ x^T -> G0/N0
L1 lags L0 by one group; double-buffered via even/odd parity tiles.
"""

import sys

sys.path.insert(0, "/opt/trn_rl_repo")

import numpy as np

import concourse.bass as bass
import concourse.bacc as bacc_mod
import concourse.mybir as mybir
from concourse.bass import ds
from concourse.tile import TileContext

B, T_FULL, I, H, G = 64, 4096, 64, 256, 768
NCORES = 8
BL = B // NCORES  # 8 sequences per core
GRP = 16          # time steps per group
C = GRP * BL      # 128 columns per group (col = t*8 + b)
NJ = 6            # gate chunks of 128 (r: 0-1, z: 2-3, n: 4-5)
NI = 2            # hidden chunks of 128
F32 = mybir.dt.float32
F32R = mybir.dt.float32r
BF16 = mybir.dt.bfloat16

SIG = mybir.ActivationFunctionType.Sigmoid
TANH = mybir.ActivationFunctionType.Tanh
MULT = mybir.AluOpType.mult
ADD = mybir.AluOpType.add
SUB = mybir.AluOpType.subtract


def _r(ap):
    return ap.bitcast(F32R)


def build_nc(T=T_FULL, unroll_all=False, debug=False):
    """unroll_all=True builds a fully python-unrolled program (for sim)."""
    nc = bacc_mod.Bacc()

    NG = T // GRP
    RPAD_IN = (NG + 2) * C    # y/z rows incl. 2 pad groups
    RPAD_OUT = (NG + 1) * C   # out rows incl. 1 pad group (dropped by host)

    y_d = nc.declare_dram_parameter("y", [RPAD_IN, I], F32R, isOutput=False)
    z_d = nc.declare_dram_parameter("z", [RPAD_IN, I], F32, isOutput=False)
    whh0_d = nc.declare_dram_parameter("whh0", [128, NJ, NI, 128], BF16,
                                       isOutput=False)
    whh1_d = nc.declare_dram_parameter("whh1", [128, NJ, NI, 128], BF16,
                                       isOutput=False)
    wih1_d = nc.declare_dram_parameter("wih1", [128, NJ, NI, 128], BF16,
                                       isOutput=False)
    wih0a_d = nc.declare_dram_parameter("wih0a", [I + 1, NJ, 128], BF16,
                                        isOutput=False)
    wlin_d = nc.declare_dram_parameter("wlin", [128, NI, I], BF16,
                                       isOutput=False)
    bias1_d = nc.declare_dram_parameter("bias1", [1, NJ, 128], BF16,
                                        isOutput=False)
    blin_d = nc.declare_dram_parameter("blin", [1, I], BF16, isOutput=False)
    bc0_d = nc.declare_dram_parameter("bc0", [128, 2, BL], BF16,
                                      isOutput=False)
    bc1_d = nc.declare_dram_parameter("bc1", [128, 2, BL], BF16,
                                      isOutput=False)
    eyef_d = nc.declare_dram_parameter("eyef", [128, 128], F32,
                                       isOutput=False)
    eyeb_d = nc.declare_dram_parameter("eyeb", [128, 128], BF16,
                                       isOutput=False)
    eyer_d = nc.declare_dram_parameter("eyer", [128, 128], F32R,
                                       isOutput=False)
    out_d = nc.declare_dram_parameter("out", [RPAD_OUT, I], F32,
                                      isOutput=True)
    h1dbg_d = h2dbg_d = None
    if debug:
        h1dbg_d = nc.declare_dram_parameter("h1dbg", [RPAD_OUT, NI, 128],
                                            BF16, isOutput=True)
        h2dbg_d = nc.declare_dram_parameter("h2dbg", [RPAD_OUT, NI, 128],
                                            BF16, isOutput=True)

    with TileContext(nc) as tc:
        with (
            tc.tile_pool(name="wpool", bufs=1) as wpool,
            tc.tile_pool(name="gatep", bufs=3) as gatep,
            tc.tile_pool(name="iop", bufs=2) as iop,
            tc.tile_pool(name="ps_a0", bufs=1, space="PSUM") as ps_a0,
            tc.tile_pool(name="ps_n0", bufs=1, space="PSUM") as ps_n0,
            tc.tile_pool(name="ps_a1", bufs=1, space="PSUM") as ps_a1,
            tc.tile_pool(name="ps_n1", bufs=1, space="PSUM") as ps_n1,
            tc.tile_pool(name="ps_gx", bufs=2, space="PSUM") as ps_gx,
            tc.tile_pool(name="ps_tp", bufs=1, space="PSUM") as ps_tp,
            tc.tile_pool(name="ps_fin", bufs=1, space="PSUM") as ps_fin,
        ):
            # ---- persistent weights / constants ----
            whh0_t = wpool.tile([128, NJ, NI, 128], BF16)
            whh1_t = wpool.tile([128, NJ, NI, 128], BF16)
            wih1_t = wpool.tile([128, NJ, NI, 128], BF16)
            wih0a_t = wpool.tile([I + 1, NJ, 128], BF16)
            wlin_t = wpool.tile([128, NI, I], BF16)
            bias1_t = wpool.tile([1, NJ, 128], BF16)
            blin_t = wpool.tile([1, I], BF16)
            bc0_t = wpool.tile([128, 2, BL], BF16)
            bc1_t = wpool.tile([128, 2, BL], BF16)
            eyef_t = wpool.tile([128, 128], F32)
            eyeb_t = wpool.tile([128, 128], BF16)
            eyer_t = wpool.tile([128, 128], F32R)
            ones1_t = wpool.tile([1, 128], BF16)
            onesf_t = wpool.tile([1, 128], F32)

            nc.sync.dma_start(out=whh0_t, in_=whh0_d[:])
            nc.sync.dma_start(out=whh1_t, in_=whh1_d[:])
            nc.sync.dma_start(out=wih1_t, in_=wih1_d[:])
            nc.sync.dma_start(out=wih0a_t, in_=wih0a_d[:])
            nc.sync.dma_start(out=wlin_t, in_=wlin_d[:])
            nc.sync.dma_start(out=bias1_t, in_=bias1_d[:])
            nc.sync.dma_start(out=blin_t, in_=blin_d[:])
            nc.sync.dma_start(out=bc0_t, in_=bc0_d[:])
            nc.sync.dma_start(out=bc1_t, in_=bc1_d[:])
            nc.sync.dma_start(out=eyef_t, in_=eyef_d[:])
            nc.sync.dma_start(out=eyeb_t, in_=eyeb_d[:])
            nc.sync.dma_start(out=eyer_t, in_=eyer_d[:])
            nc.gpsimd.memset(onesf_t[:], 1.0)
            nc.vector.tensor_copy(ones1_t[:], onesf_t[:])

            # ---- parity-pair state tiles ----
            # per parity p: G/N gx tiles, H group tiles, xTa, y/z input tiles
            G0_ = [wpool.tile([128, 4, GRP, BL], F32R, name=f"G0_{p}")
                   for p in range(2)]
            N0_ = [wpool.tile([128, 2, GRP, BL], F32, name=f"N0_{p}")
                   for p in range(2)]
            G1_ = [wpool.tile([128, 4, GRP, BL], F32R, name=f"G1_{p}")
                   for p in range(2)]
            N1_ = [wpool.tile([128, 2, GRP, BL], F32, name=f"N1_{p}")
                   for p in range(2)]
            H1_ = [wpool.tile([128, NI, C], BF16, name=f"H1_{p}")
                   for p in range(2)]
            H2_ = [wpool.tile([128, NI, C], BF16, name=f"H2_{p}")
                   for p in range(2)]
            xTa_ = [wpool.tile([I + 1, 128], BF16, name=f"xTa_{p}")
                    for p in range(2)]
            y_ = [wpool.tile([128, I], F32R, name=f"y_{p}") for p in range(2)]
            z_ = [wpool.tile([128, I], F32, name=f"z_{p}") for p in range(2)]

            # init: ones rows of xTa; zero carries and L1(-1)/final(-1) inputs
            nc.vector.tensor_copy(xTa_[0][I: I + 1, :], onesf_t[:])
            nc.vector.tensor_copy(xTa_[1][I: I + 1, :], onesf_t[:])
            nc.vector.memset(H1_[1][:], 0.0)
            nc.vector.memset(H2_[0][:], 0.0)
            nc.vector.memset(H2_[1][:], 0.0)
            nc.vector.memset(G1_[1][:].bitcast(F32), 0.0)
            nc.vector.memset(N1_[1][:], 0.0)
            nc.vector.memset(y_[1][:].bitcast(F32), 0.0)

            def gstep(l, k, par):
                """One GRU step. l=0: layer0 group g (parity par);
                l=1: layer1 group g-1 (parity 1-par)."""
                if l == 0:
                    Gt, Nt, Ht = G0_[par], N0_[par], H1_[par]
                    Hprev = H1_[1 - par]
                    W, Bc = whh0_t, bc0_t
                    psA, psN = ps_a0, ps_n0
                else:
                    Gt, Nt, Ht = G1_[1 - par], N1_[1 - par], H2_[1 - par]
                    Hprev = H2_[par]
                    W, Bc = whh1_t, bc1_t
                    psA, psN = ps_a1, ps_n1

                if k == 0:
                    hp = Hprev[:, :, C - BL: C]
                else:
                    hp = Ht[:, :, (k - 1) * BL: k * BL]

                A = psA.tile([128, 4, BL], F32, tag=f"A{l}")
                N = psN.tile([128, 2, BL], F32, tag=f"N{l}")
                for j in range(4):
                    for i in range(NI):
                        nc.tensor.matmul(A[:, j, :], W[:, j, i, :],
                                         hp[:, i, :],
                                         start=(j == 0 and i == 0),
                                         stop=False, skip_group_check=True)
                # fold gx_rz (incl. all rz biases) into PSUM
                nc.tensor.matmul(A[:, :, :], eyer_t, Gt[:, :, k, :],
                                 start=False, stop=True,
                                 skip_group_check=True)
                for j in range(2):
                    for i in range(NI):
                        nc.tensor.matmul(N[:, j, :], W[:, 4 + j, i, :],
                                         hp[:, i, :],
                                         start=(j == 0 and i == 0),
                                         stop=False, skip_group_check=True)
                # fold b_hh_n into PSUM
                nc.tensor.matmul(N[:, :, :], eyeb_t, Bc[:],
                                 start=False, stop=True,
                                 skip_group_check=True)

                S = gatep.tile([128, 4, BL], F32, tag=f"S{l}")
                nc.scalar.activation(S, A, SIG)
                m = gatep.tile([128, 2, BL], F32, tag=f"m{l}")
                nc.vector.tensor_tensor(m, N, S[:, 0:2, :], MULT)
                t = gatep.tile([128, 2, BL], F32, tag=f"t{l}")
                nc.vector.tensor_tensor(t, m, Nt[:, :, k, :], ADD)
                nt = gatep.tile([128, 2, BL], F32, tag=f"nt{l}")
                nc.scalar.activation(nt, t, TANH)
                # off-chain: zm1 = z - 1 ; c = z * h
                zm1 = gatep.tile([128, 2, BL], F32, tag=f"zm1{l}")
                nc.gpsimd.tensor_scalar(zm1, S[:, 2:4, :], 1.0, None, SUB)
                c = gatep.tile([128, 2, BL], F32, tag=f"c{l}")
                nc.gpsimd.tensor_tensor(c, S[:, 2:4, :], hp.bitcast(BF16),
                                        MULT)
                # chain: tmp = nt * zm1 ; h' = c - tmp
                tmp = gatep.tile([128, 2, BL], F32, tag=f"tmp{l}")
                nc.gpsimd.tensor_tensor(tmp, nt, zm1, MULT)
                nc.vector.tensor_tensor(Ht[:, :, k * BL: (k + 1) * BL],
                                        c, tmp, SUB)

            def gx_copy(j, dst, gps):
                """PSUM->SBUF gx copy, spread across engines by chunk."""
                if j in (0, 1, 2):
                    nc.vector.tensor_copy(dst, gps)
                else:
                    nc.scalar.copy(dst, gps)

            def bulk0(r_y, par):
                """Load y,z for a group into parity `par`, compute x^T and
                gx0 -> G0_[par], N0_[par]."""
                nc.sync.dma_start(out=y_[par], in_=y_d[ds(r_y, C), :])
                nc.sync.dma_start(out=z_[par], in_=z_d[ds(r_y, C), :])
                x_t = iop.tile([128, I], F32, tag="x")
                nc.vector.tensor_tensor(x_t, y_[par].bitcast(F32), z_[par], ADD)
                tp = ps_tp.tile([I, 128], F32, tag="tp")
                nc.tensor.transpose(tp, x_t, eyef_t)
                nc.scalar.copy(xTa_[par][0:I, :], tp)
                for j in range(NJ):
                    gps = ps_gx.tile([128, C], F32, tag="gx")
                    nc.tensor.matmul(gps, wih0a_t[:, j, :], xTa_[par],
                                     start=True, stop=True)
                    if j < 4:
                        dst = G0_[par][:, j, :, :]
                    else:
                        dst = N0_[par][:, j - 4, :, :]
                    gx_copy(j, dst, gps)

            def gx1(par):
                """W_ih1 @ H1(g) -> G1_[par], N1_[par]."""
                for j in range(NJ):
                    gps = ps_gx.tile([128, C], F32, tag="gx")
                    for i in range(NI):
                        nc.tensor.matmul(gps, wih1_t[:, j, i, :],
                                         H1_[par][:, i, :],
                                         start=(i == 0), stop=False)
                    nc.tensor.matmul(gps, bias1_t[:, j, :], ones1_t,
                                     start=False, stop=True)
                    if j < 4:
                        dst = G1_[par][:, j, :, :]
                    else:
                        dst = N1_[par][:, j - 4, :, :]
                    gx_copy(j, dst, gps)

            def final(r_o, par):
                """out rows [r_o, r_o+C) = y + W_lin @ H2(g-1) + b_lin.
                H2/y parity is 1-par (group g-1)."""
                fp = ps_fin.tile([128, I], F32, tag="fin")
                nc.tensor.matmul(fp, H2_[1 - par][:, 0, :], wlin_t[:, 0, :],
                                 start=True, stop=False)
                nc.tensor.matmul(fp, H2_[1 - par][:, 1, :], wlin_t[:, 1, :],
                                 start=False, stop=False)
                nc.tensor.matmul(fp, eyer_t, y_[1 - par],
                                 start=False, stop=False)
                nc.tensor.matmul(fp, ones1_t, blin_t,
                                 start=False, stop=True)
                o_t = iop.tile([128, I], F32, tag="o")
                nc.scalar.copy(o_t, fp)
                nc.sync.dma_start(out=out_d[ds(r_o, C), :], in_=o_t)

            def body(r0, par, first=False):
                for k in range(GRP):
                    gstep(0, k, par)
                    if not first:
                        gstep(1, k, par)
                if debug:
                    # H1 is group g at rows r0; H2 is group g-1 (host shifts)
                    nc.sync.dma_start(
                        out=h1dbg_d[ds(r0, C), :, :],
                        in_=H1_[par].bitcast(BF16))
                    nc.sync.dma_start(
                        out=h2dbg_d[ds(r0, C), :, :],
                        in_=H2_[1 - par].bitcast(BF16))
                gx1(par)
                final(r0, par)
                bulk0(r0 + C, 1 - par)

            # prologue: group 0 inputs -> parity 0
            bulk0(0, 0)

            if unroll_all:
                for g in range(NG + 1):
                    body(g * C, g % 2, first=(g == 0))
            else:
                body(0, 0, first=True)
                with tc.For_i(C, (NG + 1) * C, 2 * C,
                              staggered_reset=True) as iv:
                    body(iv, 1)
                    body(iv + C, 0)

    nc.compile()
    return nc


def prep_weights(W_ih0, W_hh0, b_ih0, b_hh0, W_ih1, W_hh1, b_ih1, b_hh1,
                 W_lin, b_lin):
    """Host-side weight folding into gate-major bf16 layouts."""
    import ml_dtypes
    bf = ml_dtypes.bfloat16
    f = np.float32

    def whh_lay(W):  # [768, 256] -> [128, 6, 2, 128]
        return np.ascontiguousarray(
            W.reshape(NJ, 128, NI, 128).transpose(3, 0, 2, 1)).astype(bf)

    bias0 = (b_ih0 + np.concatenate([b_hh0[: 2 * H], np.zeros(H, f)])).astype(f)
    bias1 = (b_ih1 + np.concatenate([b_hh1[: 2 * H], np.zeros(H, f)])).astype(f)

    # wih0a: [65, 6, 128]; rows 0:64 = W_ih0^T gate-major, row 64 = bias0
    wih0a = np.zeros((I + 1, NJ, 128), f)
    wih0a[:I] = W_ih0.reshape(NJ, 128, I).transpose(2, 0, 1)
    wih0a[I] = bias0.reshape(NJ, 128)

    bc = lambda bh: np.broadcast_to(
        bh[2 * H:].reshape(2, 128).T[:, :, None], (128, 2, BL))

    return {
        "whh0": whh_lay(W_hh0),
        "whh1": whh_lay(W_hh1),
        "wih1": whh_lay(W_ih1),
        "wih0a": wih0a.astype(bf),
        "wlin": np.ascontiguousarray(
            W_lin.T.reshape(NI, 128, I).transpose(1, 0, 2)).astype(bf),
        "bias1": bias1.reshape(1, NJ, 128).astype(bf),
        "blin": b_lin.reshape(1, I).astype(bf),
        "bc0": np.ascontiguousarray(bc(b_hh0)).astype(bf),
        "bc1": np.ascontiguousarray(bc(b_hh1)).astype(bf),
        "eyef": np.eye(128, dtype=f),
        "eyeb": np.eye(128, dtype=bf),
        "eyer": np.eye(128, dtype=f),
    }


def prep_seq(a, T):
    """[BLc, T, I] f32 -> padded [RPAD_IN, I] rows (t*BLc+b order)."""
    BLc = a.shape[0]
    NG = T // GRP
    r = np.ascontiguousarray(a.transpose(1, 0, 2)).reshape(T * BLc, I)
    pad = np.zeros(((NG + 2) * GRP * BLc - T * BLc, I), np.float32)
    return np.concatenate([r, pad], axis=0)


def unprep_out(o, T):
    """[RPAD_OUT, I] -> [BL, T, I] (drop first pad group)."""
    o = o[C:].reshape(T, BL, I)
    return np.ascontiguousarray(o.transpose(1, 0, 2))


_NC_CACHE = {}


def kernel(z, y, W_ih0, W_hh0, b_ih0, b_hh0, W_ih1, W_hh1, b_ih1, b_hh1,
           W_lin, b_lin, _trace=False):
    """Full-input entry point: shards over 8 cores, returns full output."""
    from concourse.bass_utils import run_bass_kernel_spmd

    z = np.asarray(z, np.float32)
    y = np.asarray(y, np.float32)
    T = z.shape[1]
    if T not in _NC_CACHE:
        _NC_CACHE[T] = build_nc(T=T)
    nc = _NC_CACHE[T]

    wmaps = prep_weights(
        np.asarray(W_ih0), np.asarray(W_hh0), np.asarray(b_ih0),
        np.asarray(b_hh0), np.asarray(W_ih1), np.asarray(W_hh1),
        np.asarray(b_ih1), np.asarray(b_hh1), np.asarray(W_lin),
        np.asarray(b_lin))

    in_maps = []
    for cid in range(NCORES):
        sl = slice(cid * BL, (cid + 1) * BL)
        m = {"y": prep_seq(y[sl], T), "z": prep_seq(z[sl], T)}
        m.update(wmaps)
        in_maps.append(m)

    res = run_bass_kernel_spmd(nc, in_maps, list(range(NCORES)),
                               trace=_trace)
    outs = [unprep_out(res.results[cid]["out"], T) for cid in range(NCORES)]
    full = np.concatenate(outs, axis=0).astype(np.float32)
    if _trace:
        return full, res
    return full
